# revision 75
# baseline (speedup 1.0000x reference)
"""GQA prefill kernel for 8 Trainium2 NeuronCores (software-pipelined).

Problem: B=2, T=2048, C=2048, H=32 q-heads, HKV=8 kv-heads, DH=64,
causal attention with RoPE, torch-Linear-style projections.

Sharding: core = b*4 + g over (batch b in 0..1, head-group g in 0..3).
Each core owns 8 q-heads / 2 kv-heads of one batch element:
  - Wq column-shard   -> qT   [512, T]  (features on partitions)
  - Wkv column-shard  -> kT,vT[128, T]
  - Wo row-shard      -> partial output [T, C]; host sums 4 partials/batch.

Design (vs a phase-serial implementation):
  - One software pipeline over j (512-wide q/t blocks): the attention of
    block j is emitted with the QKV projections of block j+1 interleaved
    into the PE stream as "fill" units, so the tensor engine stays busy
    while the activation engine runs the softmax exps.  All output
    projections are deferred to block 3 (the most exp-heavy attention
    block) and the epilogue.
  - bf16 everywhere except the f32 PSUM accumulators and the softmax
    scores: matmul throughput is unchanged on TRN2, SBUF/DMA halve, and
    the causal-mask multiply gets the DVE 2x/4x fast path.  Verified
    rel-err vs the fp32 reference: 5.4e-3.
  - RoPE without any partition-shift DMAs: the host lays out the dh axis
    so each rotate-half partner lies 16 slots away inside one
    32-partition lane group, making the rotation a single DVE
    stream_shuffle (straight out of PSUM), followed by two multiplies
    with host-prebaked signed cos/sin tables.
  - Causality at 128-column granularity: diagonal score tiles compute
    only the valid q sub-range (bf16 keeps full PE rate at any width);
    masking is a 0/1 bf16 multiply on 128 columns of the exp'd probs.
  - The softmax denominator rides as a 65th ones-column of the value
    tiles (row 64 of the attn@V accumulator); normalization is
    reciprocal (DVE) + partition_broadcast (Pool) + multiply (DVE).
  - V transposed to k-major layout via DMA-transpose (2-byte path)
    instead of PE transposes.
  - Weights are stored host-side in chain-major layout so one DMA
    unblocks one projection chain; x streams in 4 KB/partition chunks.
    A dozen bf16 warm-up matmuls ramp the PE p-state while the first
    DMAs land.
"""

import itertools
import sys

sys.path.insert(0, "/opt/trn_rl_repo")

import numpy as np
import ml_dtypes

import concourse.bass as bass
import concourse.tile as tile
from concourse import bacc
from concourse import mybir
from concourse import bass_utils
from concourse.masks import make_identity

F32 = mybir.dt.float32
F32R = mybir.dt.float32r
BF16 = mybir.dt.bfloat16
AF = mybir.ActivationFunctionType

B, T, C, DH = 2, 2048, 2048, 64
NCORE = 8


def _r(ap):
    return ap.bitcast(F32R)


_ROPE_MASK = list(range(16, 32)) + list(range(0, 16))


def _roundrobin(gens):
    pending = [iter(g) for g in gens]
    while pending:
        alive = []
        for g in pending:
            u = next(g, None)
            if u is not None:
                yield u
                alive.append(g)
        pending = alive


class _Kern:
    def __init__(self, tc, io):
        self.tc = tc
        self.nc = tc.nc
        (self.xT, self.wqT, self.wkvT, self.woT, self.cosT, self.sinT,
         self.bmaskT, self.out) = io
        self.xts = {}

    # ---------- DMA issue helpers ----------
    def issue_x(self, j):
        nc = self.nc
        for cc in range(4):
            t = self.xp.tile([128, 2048], BF16, tag="x", name=f"x{j}_{cc}")
            src = self.xT[cc * 512:(cc + 1) * 512, j * 512:(j + 1) * 512]
            nc.gpsimd.dma_start(out=t[:], in_=src.rearrange(
                "(ct p) n -> p ct n", p=128))
            self.xts[(j, cc)] = t

    # ---------- generators of PE work units ----------
    def gen_A(self, jn):
        """QKV projections + RoPE + V transpose for block jn.
        Yields one callable per PE instruction; non-PE work is emitted
        inline at chain boundaries."""
        nc = self.nc
        for g in (4, 0, 5, 1, 2, 3):  # k, q0, v, q1, q2, q3
            acc = self.ps.tile([128, 512], F32, tag="accA", bufs=2,
                               name=f"accA{jn}_{g}")
            for c in range(16):
                if g < 4:
                    lh = self.wq[:, g * 2048 + c * 128: g * 2048 + (c + 1) * 128]
                elif g == 4:
                    lh = self.wkv[:, c * 128:(c + 1) * 128]
                else:
                    lh = self.wkv[:, 2048 + c * 128: 2048 + (c + 1) * 128]
                rhs = self.xts[(jn, c // 4)][:, (c % 4) * 512:(c % 4 + 1) * 512]

                def mk(acc=acc, lh=lh, rhs=rhs, c=c):
                    nc.tensor.matmul(acc[:], lhsT=lh, rhs=rhs,
                                     start=(c == 0), stop=(c == 15))
                yield mk
            if g == 5:
                vraw = self.miscp.tile([128, 512], BF16, tag="vraw", bufs=2,
                                       name=f"vraw{jn}")
                nc.vector.tensor_copy(vraw[:], acc[:])
                for tt in range(4):
                    gt = jn * 4 + tt
                    vtmp = self.miscp.tile([128, 128], BF16, tag="vtmp",
                                           bufs=6, name=f"vtmp{gt}")
                    nc.sync.dma_start(out=vtmp[:],
                                      in_=vraw[:, tt * 128:(tt + 1) * 128],
                                      transpose=True)
                    nc.gpsimd.tensor_copy(
                        self.v_aug[:, gt * 65: gt * 65 + 64],
                        vtmp[:, 0:64])
                    nc.gpsimd.tensor_copy(
                        self.v_aug[:, 1040 + gt * 65: 1040 + gt * 65 + 64],
                        vtmp[:, 64:128])
            else:
                # RoPE for a q (g<4) or k (g==4) chain.  The host lays the
                # dh axis out so each rotate-half partner sits 16 positions
                # away within the same 32-partition lane group; the swap is
                # then a single DVE stream_shuffle straight out of PSUM.
                sh = self.ropep.tile([128, 512], F32, tag="sh", bufs=2,
                                     name=f"sh{jn}_{g}")
                nc.vector.stream_shuffle(sh[:], acc[:], _ROPE_MASK)
                jc = slice(jn * 512, (jn + 1) * 512)
                dst = (self.qT[:, g * 2048 + jn * 512:
                               g * 2048 + (jn + 1) * 512]
                       if g < 4 else self.kT[:, jc])
                tmpc = self.ropep.tile([128, 512], F32, tag="tmpc", bufs=2,
                                       name=f"tmpc{jn}_{g}")
                tmps = self.ropep.tile([128, 512], F32, tag="tmps", bufs=2,
                                       name=f"tmps{jn}_{g}")
                nc.vector.tensor_mul(tmpc[:], acc[:], self.cos_sb[:, jc])
                nc.vector.tensor_mul(tmps[:], sh[:], self.sin_sb[:, jc])
                nc.vector.tensor_add(dst, tmpc[:], tmps[:])

    def gen_D(self, jo, tag="accD", epilogue=False):
        """Output projection for t-block jo (needs aT of block jo)."""
        nc = self.nc
        for tt in range(jo * 4, jo * 4 + 4):
            ost = self.miscp.tile([128, 2048], BF16, tag="ost", bufs=3,
                                  name=f"ost{tt}")
            for cb in range(4):
                acc = self.ps.tile([128, 512], F32, tag=tag, bufs=2,
                                   name=f"od{tt}_{cb}")
                for f in range(4):
                    def mk(acc=acc, tt=tt, cb=cb, f=f):
                        nc.tensor.matmul(
                            acc[:],
                            lhsT=self.aT[:, f * 2048 + tt * 128:
                                         f * 2048 + tt * 128 + 128],
                            rhs=self.wo[:, f * 2048 + cb * 512:
                                        f * 2048 + (cb + 1) * 512],
                            start=(f == 0), stop=(f == 3))
                    yield mk
                nc.vector.tensor_copy(ost[:, cb * 512:(cb + 1) * 512], acc[:])
                if epilogue and tt >= 14:
                    # pipeline the very last stores per 512 columns so the
                    # final DMA starts as early as possible
                    nc.sync.dma_start(
                        out=self.out[tt * 128:(tt + 1) * 128,
                                     cb * 512:(cb + 1) * 512],
                        in_=ost[:, cb * 512:(cb + 1) * 512])
                elif cb % 2:
                    h = cb // 2
                    nc.sync.dma_start(
                        out=self.out[tt * 128:(tt + 1) * 128,
                                     h * 1024:(h + 1) * 1024],
                        in_=ost[:, h * 1024:(h + 1) * 1024])

    # ---------- attention over one j block ----------
    def run_C(self, j, fills):
        nc = self.nc
        nk = 4 * j + 4

        def pull(k):
            for _ in range(k):
                u = next(fills, None)
                if u is not None:
                    u()

        for hv, d0 in ((0, 0), (0, 2), (1, 0), (1, 2)):
            pav = self.ps.tile([65, 1024], F32, tag="pav", bufs=1,
                               name=f"pav{j}_{hv}_{d0}")
            for i in range(nk):
                m = i - 4 * j
                # causal: for diagonal tiles only q columns >= 128*m can
                # see k tile i; bf16 matmuls run 1 cy/row at any width.
                q0 = max(m, 0) * 128
                w = 512 - q0
                sc_h = []
                for h in (0, 1):
                    d = d0 + h
                    sc = self.ps.tile([128, 512], F32, tag="sc", bufs=4,
                                      name=f"sc{j}_{hv}_{d}_{i}")
                    nc.tensor.matmul(
                        sc[:, 0:w],
                        lhsT=self.kT[hv * 64:hv * 64 + 64,
                                     i * 128:(i + 1) * 128],
                        rhs=self.qT[hv * 64:hv * 64 + 64,
                                    d * 2048 + j * 512 + q0:
                                    d * 2048 + (j + 1) * 512],
                        start=True, stop=True)
                    sc_h.append(sc)
                pr_h = []
                for h in (0, 1):
                    pr = self.probsp.tile([128, 512], BF16, tag="pr", bufs=6,
                                          name=f"pr{j}_{hv}_{d0 + h}_{i}")
                    nc.scalar.activation(pr[:, 0:w], sc_h[h][:, 0:w], AF.Exp,
                                         scale=0.125)
                    if m >= 0:
                        # only the leading 128 columns of the remaining
                        # range are partially masked (the triangle)
                        nc.vector.tensor_mul(
                            pr[:, 0:128], pr[:, 0:128],
                            self.bmask[:, m * 512 + q0:m * 512 + q0 + 128])
                    pr_h.append(pr)
                if i >= 6 or hv + d0 > 0:
                    pull(2)
                for h in (0, 1):
                    nc.tensor.matmul(
                        pav[:, h * 512 + q0:(h + 1) * 512],
                        lhsT=self.v_aug[:, hv * 1040 + i * 65:
                                        hv * 1040 + i * 65 + 65],
                        rhs=pr_h[h][:, 0:w],
                        start=(i == 0), stop=(i == nk - 1),
                        skip_group_check=True)
            # normalize and write aT (bf16)
            bc = self.miscp.tile([64, 1024], F32, tag="bc", bufs=2,
                                 name=f"bc{j}_{hv}_{d0}")
            for h in (0, 1):
                den = self.miscp.tile([1, 512], F32, tag="den", bufs=2,
                                      name=f"den{j}_{hv}_{d0}_{h}")
                nc.vector.reciprocal(den[:], pav[64:65,
                                                 h * 512:(h + 1) * 512])
                nc.gpsimd.partition_broadcast(bc[:, h * 512:(h + 1) * 512],
                                              den[:])
            for h in (0, 1):
                d = d0 + h
                nc.vector.tensor_mul(
                    self.aT[hv * 64:hv * 64 + 64,
                            d * 2048 + j * 512: d * 2048 + (j + 1) * 512],
                    pav[0:64, h * 512:(h + 1) * 512],
                    bc[:, h * 512:(h + 1) * 512])
            pull(8)

    # ---------- full kernel ----------
    def build(self):
        nc = self.nc
        tc = self.tc
        with tc.tile_pool(name="cst", bufs=1) as cst, \
             tc.tile_pool(name="xp", bufs=5) as self.xp, \
             tc.tile_pool(name="ropep", bufs=2) as self.ropep, \
             tc.tile_pool(name="probsp", bufs=4) as self.probsp, \
             tc.tile_pool(name="miscp", bufs=2) as self.miscp, \
             tc.tile_pool(name="ps", bufs=1, space="PSUM") as self.ps:
            self.wq = cst.tile([128, 16 * 512], BF16, name="wq")
            self.wkv = cst.tile([128, 16 * 256], BF16, name="wkv")
            self.qT = cst.tile([128, 4 * 2048], BF16, name="qT")
            self.kT = cst.tile([128, 2048], BF16, name="kT")
            self.v_aug = cst.tile([128, 2 * 16 * 65], BF16, name="v_aug")
            self.aT = cst.tile([128, 4 * 2048], BF16, name="aT")
            self.wo = cst.tile([128, 4 * 2048], BF16, name="wo")
            self.cos_sb = cst.tile([128, T], BF16, name="cos_sb")
            self.sin_sb = cst.tile([128, T], BF16, name="sin_sb")
            self.bmask = cst.tile([128, 2048], BF16, name="bmask")
            self.ident = cst.tile([128, 128], F32, name="ident")
            warm = cst.tile([128, 256], BF16, name="warm")

            # --- prologue: small compute + all early DMAs ---
            nc.vector.memset(warm[:], 0.0)
            make_identity(nc, self.ident[:])
            for hv in range(2):
                for gt in range(16):
                    o = hv * 1040 + gt * 65 + 64
                    nc.vector.memset(self.v_aug[:, o:o + 1], 1.0)

            # weight/x DMAs ordered by first use; weights are stored
            # host-side in chain-major layout so ONE DMA unblocks a chain.
            def wkv_dma(h):
                nc.sync.dma_start(
                    out=self.wkv[:, h * 2048:(h + 1) * 2048],
                    in_=self.wkvT[h * 128:(h + 1) * 128, :])
            def wq_dma(g):
                nc.sync.dma_start(
                    out=self.wq[:, g * 2048:(g + 1) * 2048],
                    in_=self.wqT[g * 128:(g + 1) * 128, :])
            def x_dma(cc):
                t = self.xp.tile([128, 2048], BF16, tag="x", name=f"x0_{cc}")
                s = self.xT[cc * 512:(cc + 1) * 512, 0:512]
                nc.sync.dma_start(out=t[:], in_=s.rearrange(
                    "(ct p) n -> p ct n", p=128))
                self.xts[(0, cc)] = t
            def cs_dma(q, eng=None):
                eng = eng or nc.scalar
                sl = slice(q * 512, (q + 1) * 512)
                eng.dma_start(out=self.cos_sb[:, sl], in_=self.cosT[:, sl])
                eng.dma_start(out=self.sin_sb[:, sl], in_=self.sinT[:, sl])
            wkv_dma(0)
            x_dma(0)
            wq_dma(0)
            x_dma(1)
            cs_dma(0, eng=nc.sync)
            x_dma(2)
            wkv_dma(1)
            x_dma(3)
            nc.sync.dma_start(out=self.bmask[:], in_=self.bmaskT[:])
            wq_dma(1)
            wq_dma(2)
            wq_dma(3)

            # warm-up matmuls: ramp the PE while DMAs land (bf16, cheap,
            # no dependency beyond the DVE memset)
            for w in range(14):
                wacc = self.ps.tile([128, 512], F32, tag="accA", bufs=2,
                                    name=f"warm{w}")
                nc.tensor.matmul(wacc[:, 0:256], lhsT=warm[:, 0:128],
                                 rhs=warm[:], start=True, stop=True)

            # block-0 projections (nothing to interleave yet)
            for u in self.gen_A(0):
                u()

            # --- main pipeline over j ---
            for j in range(4):
                if j == 0:
                    for q in range(1, 4):
                        cs_dma(q)
                if j == 1:
                    # keep the big wo transfer out of the prologue's
                    # critical x/weight window: the DMA must wait for a
                    # read of its target region whose other operand only
                    # becomes ready once the x(0) stream has landed
                    gd = self.miscp.tile([1, 1], F32, tag="gate", bufs=1,
                                         name="wo_gate")
                    nc.gpsimd.tensor_tensor(
                        out=gd[:], in0=self.wo[0:1, 0:1],
                        in1=self.xts[(0, 3)][0:1, 0:1],
                        op=mybir.AluOpType.add)
                    nc.scalar.dma_start(out=self.wo[:],
                                        in_=self.woT.rearrange(
                                            "(f p) n -> p f n", p=128))
                if j < 3:
                    self.issue_x(j + 1)
                gens = []
                if j == 3:
                    gens.append(itertools.chain(
                        self.gen_D(0, tag="accA"),
                        self.gen_D(1, tag="accA"),
                        self.gen_D(2, tag="accA")))
                if j < 3:
                    gens.append(self.gen_A(j + 1))
                fills = _roundrobin(gens)
                self.run_C(j, fills)
                for u in fills:
                    u()

            # epilogue: last output-projection block
            for u in self.gen_D(3, tag="accA", epilogue=True):
                u()


_cached_nc = None


def _build():
    global _cached_nc
    if _cached_nc is not None:
        return _cached_nc
    nc = bacc.Bacc("TRN2", target_bir_lowering=False, debug=False,
                   num_devices=NCORE)
    io = (
        nc.dram_tensor("xT", [C, T], BF16, kind="ExternalInput").ap(),
        nc.dram_tensor("wqT", [512, C], BF16, kind="ExternalInput").ap(),
        nc.dram_tensor("wkvT", [256, C], BF16, kind="ExternalInput").ap(),
        nc.dram_tensor("woT", [512, C], BF16, kind="ExternalInput").ap(),
        nc.dram_tensor("cosT", [128, T], BF16, kind="ExternalInput").ap(),
        nc.dram_tensor("sinT", [128, T], BF16, kind="ExternalInput").ap(),
        nc.dram_tensor("bmaskT", [128, 2048], BF16,
                       kind="ExternalInput").ap(),
        nc.dram_tensor("out", [T, C], BF16, kind="ExternalOutput").ap(),
    )
    with tile.TileContext(nc) as tc:
        with nc.allow_low_precision(reason="bf16 attention operands"):
            _Kern(tc, io).build()
    nc.compile()
    _cached_nc = nc
    return nc


def _prep_in_maps(x, cos, sin, Wq, Wkv, Wo):
    x = np.asarray(x, np.float32)
    cos = np.asarray(cos, np.float32)
    sin = np.asarray(sin, np.float32)
    Wq = np.asarray(Wq, np.float32)
    Wkv = np.asarray(Wkv, np.float32)
    Wo = np.asarray(Wo, np.float32)

    p = np.arange(128)
    # dh layout within each 64-wide head: rotate-half partners (dh, dh+32)
    # are placed 16 apart inside one 32-partition lane group, so the swap
    # is a DVE stream_shuffle.  dhmap[b] = original dh stored at slot b.
    b = np.arange(64)
    dhmap = np.where(b < 16, b,
                     np.where(b < 32, b + 16,
                              np.where(b < 48, b - 16, b)))
    p_dh = dhmap[p % 64]
    cosT = np.ascontiguousarray(
        cos[:, p_dh % 32].T).astype(ml_dtypes.bfloat16)    # [128, T]
    sgn = np.where(p_dh < 32, -1.0, 1.0).astype(np.float32)
    sinT = np.ascontiguousarray(
        sin[:, p_dh % 32].T * sgn[:, None]).astype(ml_dtypes.bfloat16)
    n = np.arange(512)
    bmaskT = np.empty((128, 2048), np.float32)
    for m in range(4):
        bmaskT[:, m * 512:(m + 1) * 512] = (
            (128 * m + p)[:, None] <= n[None, :]).astype(np.float32)
    bmaskT = bmaskT.astype(ml_dtypes.bfloat16)

    qperm = np.empty(512, np.int64)
    operm = np.empty(512, np.int64)
    for dd_t in range(4):
        for o in (0, 64):
            hq = dd_t + (o // 64) * 4
            qperm[dd_t * 128 + o: dd_t * 128 + o + 64] = hq * 64 + dhmap
            operm[dd_t * 128 + o: dd_t * 128 + o + 64] = \
                np.arange(hq * 64, hq * 64 + 64)

    in_maps = []
    for b in range(B):
        xTb = np.ascontiguousarray(x[b].T).astype(ml_dtypes.bfloat16)
        for g in range(4):
            wqT0 = Wq[g * 512:(g + 1) * 512, :][qperm].T  # [2048, 512]
            wqT = np.ascontiguousarray(
                wqT0.reshape(16, 128, 4, 128).transpose(2, 1, 0, 3)
                .reshape(512, 2048)).astype(ml_dtypes.bfloat16)
            krows = Wkv[128 * g:128 * g + 128]
            kperm = np.concatenate([dhmap, 64 + dhmap])
            wkvT0 = np.concatenate(
                [krows[kperm],
                 Wkv[512 + 128 * g:512 + 128 * g + 128]], 0).T  # [2048, 256]
            wkvT = np.ascontiguousarray(
                wkvT0.reshape(16, 128, 2, 128).transpose(2, 1, 0, 3)
                .reshape(256, 2048)).astype(ml_dtypes.bfloat16)
            woT = np.ascontiguousarray(
                Wo[:, g * 512:(g + 1) * 512].T[operm]).astype(
                    ml_dtypes.bfloat16)
            in_maps.append({"xT": xTb, "wqT": wqT, "wkvT": wkvT, "woT": woT,
                            "cosT": cosT, "sinT": sinT, "bmaskT": bmaskT})
    return in_maps


def _run(x, cos, sin, Wq, Wkv, Wo, trace=False):
    nc = _build()
    in_maps = _prep_in_maps(x, cos, sin, Wq, Wkv, Wo)
    res = bass_utils.run_bass_kernel_spmd(nc, in_maps,
                                          core_ids=list(range(NCORE)),
                                          trace=trace)
    out = np.zeros((B, T, C), np.float32)
    for b in range(B):
        for g in range(4):
            out[b] += res.results[b * 4 + g]["out"].astype(np.float32)
    return out, res


def kernel(x, cos, sin, Wq, Wkv, Wo):
    out, _ = _run(x, cos, sin, Wq, Wkv, Wo)
    return out
